# revision 20
# baseline (speedup 1.0000x reference)
"""Multi-head causal attention (B=2, S=2048, D=1024, H=16) on 8 TRN2 NeuronCores.

Sharding: core c -> (head-group g = c//2 of 4 heads, batch half s = c%2).
Each core computes Q/K/V projections for its 4 heads over its batch element,
causal softmax attention, and a partial output projection (its 256 columns of
Wo). Host sums the 4 per-group partials for each batch element and adds bo.

Device layout notes:
- All activation-side tensors are bf16 (full-rate on the PE, ~4e-3 rel err).
- Activations X are passed pre-transposed (X^T, [D, S]) so every projection
  contracts over the embed dim on the partition axis.
- Scores are computed transposed (S^T [k, q]) so the attention matmul
  (attn @ V) needs no transposes; softmax denominators come from an
  appended ones-column in V, and the normalization divide uses a K=1
  broadcast matmul + vector reciprocal.
- The causal mask on diagonal 128-blocks is applied as a -1e30 PSUM pre-bias
  written by a tiny PE matmul (identity x triangle) before the QK matmuls,
  so the score->exp->attnV chain never touches the vector engine.
- DMAs are split across both HWDGE rings (SP: xq/xk/wq/wk, Act: xv/weights/
  constants/output) to double effective DMA bandwidth.
- PSUM tags: "proj" (2 banks, Q/K chains), "sc" (4 banks, V-proj + scores +
  denom broadcast), "acc" (2 banks, attnV accumulators + out-proj), so a
  next loop iteration's projections don't serialize behind this iteration's
  attention tail.
"""

import contextlib
import sys

sys.path.insert(0, "/opt/trn_rl_repo")

import numpy as np

import concourse.bass as bass  # noqa: F401  (bass must import before bacc)
import concourse.mybir as mybir
from concourse import bacc
from concourse.bass_utils import run_bass_kernel_spmd
from concourse.tile import TileContext

F32 = mybir.dt.float32
F32R = mybir.dt.float32r
BF16 = mybir.dt.bfloat16
AF = mybir.ActivationFunctionType
ALU = mybir.AluOpType

B = 2
S = 2048            # sequence per batch element (= rows per core)
D = 1024            # embed dim
H = 16              # total heads
HD = 64             # head dim
DL = 256            # local dims per core (4 heads)
NI = D // 128       # 8 contraction tiles for projections
NQ = S // 512       # 4 query tiles of 512
NK = S // 128       # 16 key tiles of 128
SCALE = HD ** -0.5
NEG = -1e30


def _build_nc(loop_iters=None, phases="full"):
    nc = bacc.Bacc()

    xq_d = nc.declare_dram_parameter("xq_t", [D, S], BF16, isOutput=False)
    xk_d = nc.declare_dram_parameter("xk_t", [D, S], BF16, isOutput=False)
    xv_d = nc.declare_dram_parameter("xv_t", [D, S], BF16, isOutput=False)
    wq_d = nc.declare_dram_parameter("wq_t", [D, DL], BF16, isOutput=False)
    wk_d = nc.declare_dram_parameter("wk_t", [D, DL], BF16, isOutput=False)
    wv_d = nc.declare_dram_parameter("wv_t", [D, DL], BF16, isOutput=False)
    wo_d = nc.declare_dram_parameter("wo_t", [DL, D], BF16, isOutput=False)
    bq_d = nc.declare_dram_parameter("bq", [DL, 1], F32, isOutput=False)
    bk_d = nc.declare_dram_parameter("bk", [DL, 1], F32, isOutput=False)
    bv_d = nc.declare_dram_parameter("bv_bc", [128, DL], F32, isOutput=False)
    id_d = nc.declare_dram_parameter("id128", [128, 128], BF16, isOutput=False)
    tri_d = nc.declare_dram_parameter("tri128", [128, 128], BF16,
                                      isOutput=False)
    on_d = nc.declare_dram_parameter("ones66", [66, 128], F32R, isOutput=False)
    oc_d = nc.declare_dram_parameter("ones_col", [128, NK, 1], BF16,
                                     isOutput=False)
    out_d = nc.declare_dram_parameter("out", [S, D], BF16, isOutput=True)

    with TileContext(nc) as tc:
        with tc.tile_pool(name="const", bufs=1) as cp, \
             tc.tile_pool(name="xpool", bufs=4) as xp, \
             tc.tile_pool(name="work", bufs=3) as wp, \
             tc.tile_pool(name="psum", bufs=8, space="PSUM") as pp:

            ET = mybir.EngineType
            loop_cm = (tc.For_i(0, loop_iters, 1,
                                hint_engines=(ET.PE, ET.DVE, ET.Activation,
                                              ET.SP, ET.Pool))
                       if loop_iters else contextlib.nullcontext())
            with loop_cm:
                # ---- persistent SBUF tensors ----
                wq_sb = cp.tile([128, NI * DL], BF16)
                wk_sb = cp.tile([128, NI * DL], BF16)
                wv_sb = cp.tile([128, NI * DL], BF16)
                wo_sb = cp.tile([128, 2 * D], BF16)
                qt_sb = cp.tile([128, 2 * S], BF16)   # Q^T: pair p cols [p*S:(p+1)*S]
                kt_sb = cp.tile([128, 2 * S], BF16)
                at_sb = cp.tile([128, 2 * S], BF16)   # attn out^T (normalized)
                va0 = cp.tile([128, NK * 65], BF16)   # head A of pair 0, +ones col 64
                va1 = cp.tile([128, NK * 65], BF16)
                vb0 = cp.tile([128, NK * 128], BF16)  # head B: col0=ones, 64:128=V
                vb1 = cp.tile([128, NK * 128], BF16)
                va = [va0, va1]
                vb = [vb0, vb1]
                id_sb = cp.tile([128, 128], BF16)
                ones_sb = cp.tile([66, 128], F32R)
                tri_sb = cp.tile([128, 128], BF16)
                bq_sb = cp.tile([128, 2], F32)
                bk_sb = cp.tile([128, 2], F32)
                bv_sb = cp.tile([128, DL], F32)

                nc.sync.dma_start(
                    out=wq_sb.rearrange("p (a m) -> p a m", m=DL)[:, 0:1],
                    in_=wq_d.rearrange("(a p) m -> p a m", p=128)[:, 0:1])
                nc.sync.dma_start(
                    out=wq_sb.rearrange("p (a m) -> p a m", m=DL)[:, 1:],
                    in_=wq_d.rearrange("(a p) m -> p a m", p=128)[:, 1:])

                # ---- phase 1: projections, emitted per jn and interleaved
                # with attention/out-proj below ----
                def project(jn, pend=None):
                    nsl = slice(jn * 512, (jn + 1) * 512)
                    xq_sl = xp.tile([128, NI * 512], BF16, tag="xq", bufs=2,
                                    name=f"xq_{jn}")
                    xk_sl = xp.tile([128, NI * 512], BF16, tag="xk", bufs=2,
                                    name=f"xk_{jn}")
                    for hf in range(2):  # half-slab DMAs: wave A starts after
                        hi = slice(hf * 4, (hf + 1) * 4)     # the first half
                        hr = slice(hf * 512, (hf + 1) * 512)
                        if jn == 0 and hf == 0:
                            nc.sync.dma_start(
                                out=xq_sl.rearrange(
                                    "p (a n) -> p a n", n=512)[:, 0:1],
                                in_=xq_d[0:128, nsl].rearrange(
                                    "(a p) n -> p a n", p=128))
                            nc.sync.dma_start(
                                out=xq_sl.rearrange(
                                    "p (a n) -> p a n", n=512)[:, 1:4],
                                in_=xq_d[128:512, nsl].rearrange(
                                    "(a p) n -> p a n", p=128))
                        else:
                            nc.sync.dma_start(
                                out=xq_sl.rearrange(
                                    "p (a n) -> p a n", n=512)[:, hi],
                                in_=xq_d[hr, nsl].rearrange(
                                    "(a p) n -> p a n", p=128))
                        if jn == 0 and hf == 0:
                            # first iteration of the ring: interleave the
                            # small constants behind the first q-slab half
                            nc.sync.dma_start(
                                out=wk_sb.rearrange("p (a m) -> p a m", m=DL),
                                in_=wk_d.rearrange("(a p) m -> p a m", p=128))
                        nc.sync.dma_start(
                            out=xk_sl.rearrange("p (a n) -> p a n", n=512)[:, hi],
                            in_=xk_d[hr, nsl].rearrange("(a p) n -> p a n", p=128))
                        if jn == 0 and hf == 0:
                            for p in range(2):
                                nc.sync.dma_start(
                                    out=bq_sb[:, p:p + 1],
                                    in_=bq_d[p * 128:(p + 1) * 128, :])
                                nc.sync.dma_start(
                                    out=bk_sb[:, p:p + 1],
                                    in_=bk_d[p * 128:(p + 1) * 128, :])
                            nc.sync.dma_start(out=bv_sb, in_=bv_d[:])
                            nc.sync.dma_start(out=ones_sb, in_=on_d[:])
                    xq_t = [xq_sl[:, ji * 512:(ji + 1) * 512] for ji in range(NI)]
                    xk_t = [xk_sl[:, ji * 512:(ji + 1) * 512] for ji in range(NI)]

                    if jn == 0:
                        # V path ahead of the remaining constants so wave B
                        # isn't head-of-line blocked on them
                        nc.sync.dma_start(
                            out=wv_sb.rearrange("p (a m) -> p a m", m=DL),
                            in_=wv_d.rearrange("(a p) m -> p a m", p=128))

                    # wave A: Q/K projections as four 512-col chains on the
                    # 2-bank "proj" tag (t-halves sequential), so next-iter
                    # projections never contend with this-iter scores banks.
                    for t in range(2):
                        ps_q = pp.tile([128, 512], F32, tag="proj", bufs=2,
                                       name=f"psq_{jn}_{t}")
                        for ji in range(NI):
                            wsl = slice(ji * DL + t * 128,
                                        ji * DL + (t + 1) * 128)
                            nc.tensor.matmul(ps_q, wq_sb[:, wsl], xq_t[ji],
                                             start=ji == 0, stop=ji == NI - 1)
                        dst = slice(t * S + jn * 512, t * S + (jn + 1) * 512)
                        if t == 0 and pend is not None:
                            pend()  # previous norm finishers ride this stream
                        nc.vector.tensor_scalar(qt_sb[:, dst], ps_q,
                                                bq_sb[:, t:t + 1], None, ALU.add)
                        ps_k = pp.tile([128, 512], F32, tag="proj", bufs=2,
                                       name=f"psk_{jn}_{t}")
                        for ji in range(NI):
                            wsl = slice(ji * DL + t * 128,
                                        ji * DL + (t + 1) * 128)
                            nc.tensor.matmul(ps_k, wk_sb[:, wsl], xk_t[ji],
                                             start=ji == 0, stop=ji == NI - 1)
                        nc.vector.tensor_scalar(kt_sb[:, dst], ps_k,
                                                bk_sb[:, t:t + 1], None, ALU.add)

                    # wave B: V projection, 4 chains of 256 cols in one
                    # 2-bank "sc" tile (sibling chains share a bank via the
                    # per-element overwrite-when-bit-clear PSUM semantics).
                    xv_sl = xp.tile([128, NI * 512], BF16, tag="xv", bufs=2,
                                    name=f"xv_{jn}")
                    for hf in range(2):
                        hi = slice(hf * 4, (hf + 1) * 4)
                        hr = slice(hf * 512, (hf + 1) * 512)
                        nc.sync.dma_start(
                            out=xv_sl.rearrange("p (a n) -> p a n", n=512)[:, hi],
                            in_=xv_d[hr, nsl].rearrange("(a p) n -> p a n", p=128))
                    if jn == 0:
                        # remaining phase-2/3 constants, needed later than xv
                        nc.sync.dma_start(out=id_sb, in_=id_d[:])
                        nc.sync.dma_start(out=tri_sb, in_=tri_d[:])
                        for p in range(2):
                            nc.sync.dma_start(
                                out=va[p].rearrange(
                                    "q (m c) -> q m c", c=65)[:, :, 64:65],
                                in_=oc_d[:])
                            nc.sync.dma_start(
                                out=vb[p].rearrange(
                                    "q (m c) -> q m c", c=128)[:, :, 0:1],
                                in_=oc_d[:])
                        nc.sync.dma_start(
                            out=wo_sb.rearrange("p (a m) -> p a m", m=D),
                            in_=wo_d.rearrange("(a p) m -> p a m", p=128))
                    xv_t = [xv_sl[:, ji * 512:(ji + 1) * 512] for ji in range(NI)]
                    ps_v2 = pp.tile([128, 1024], F32, tag="sc", bufs=2,
                                    name=f"psv_{jn}")
                    ps_v = [ps_v2[:, u * DL:(u + 1) * DL] for u in range(4)]
                    for ji in range(NI):
                        sp = ji == NI - 1
                        for u in range(4):
                            nc.tensor.matmul(
                                ps_v[u],
                                xv_t[ji][:, u * 128:(u + 1) * 128],
                                wv_sb[:, ji * DL:(ji + 1) * DL],
                                start=(ji == 0 and u % 2 == 0), stop=sp)
                    for u in range(4):
                        m = jn * 4 + u
                        for p in range(2):
                            ha = slice(p * 128, p * 128 + 64)
                            hb = slice(p * 128 + 64, p * 128 + 128)
                            nc.vector.tensor_tensor(
                                out=va[p][:, m * 65:m * 65 + 64],
                                in0=ps_v[u][:, ha], in1=bv_sb[:, ha], op=ALU.add)
                            nc.vector.tensor_tensor(
                                out=vb[p][:, m * 128 + 64:m * 128 + 128],
                                in0=ps_v[u][:, hb], in1=bv_sb[:, hb], op=ALU.add)

                # ---- phase 2 + 3: causal attention (head pairs packed on
                # partitions), with the partial output projection interleaved
                # per q-tile ----
                def attention(p, jq, pend=None):
                    qsl = slice(p * S + jq * 512, p * S + (jq + 1) * 512)
                    nk = 4 * jq + 4
                    ps_oa = pp.tile([65, 512], F32, tag="acc", bufs=2,
                                    name=f"oa{p}_{jq}")
                    ps_ob = pp.tile([128, 512], F32, tag="acc", bufs=2,
                                    name=f"ob{p}_{jq}")
                    for jk in range(nk):
                        d = jk - 4 * jq
                        c0 = 128 * d if d > 0 else 0  # first causally-valid col
                        ksl = slice(p * S + jk * 128, p * S + (jk + 1) * 128)
                        qsl_v = slice(p * S + jq * 512 + c0,
                                      p * S + (jq + 1) * 512)
                        ps_s2 = pp.tile([128, 1024], F32, tag="sc", bufs=2,
                                        name=f"s2{p}_{jq}_{jk}")
                        st = d < 0
                        if d >= 0:
                            # diagonal block: pre-bias the masked triangle
                            # (rows k > cols q within the 128-window) with
                            # -1e30 via a tiny PE matmul; start=True clears
                            # the bank so the QK matmuls overwrite the rest.
                            nc.tensor.matmul(ps_s2[:, c0:c0 + 128],
                                             id_sb, tri_sb,
                                             start=True, stop=True)
                            nc.tensor.matmul(ps_s2[:, 512 + c0:512 + c0 + 128],
                                             id_sb, tri_sb,
                                             start=True, stop=True)
                        nc.tensor.matmul(ps_s2[:, c0:512], kt_sb[0:64, ksl],
                                         qt_sb[0:64, qsl_v],
                                         start=st, stop=True,
                                         tile_position=(0, 0),
                                         skip_group_check=not st)
                        nc.tensor.matmul(ps_s2[:, 512 + c0:1024],
                                         kt_sb[64:128, ksl],
                                         qt_sb[64:128, qsl_v],
                                         start=st, stop=True,
                                         tile_position=(64, 0),
                                         skip_group_check=not st)
                        e2 = wp.tile([128, 1024], BF16, tag="e2", bufs=8,
                                     name=f"e2{p}_{jq}_{jk}")
                        s2v = ps_s2.rearrange("q (h n) -> q h n", n=512)[:, :, c0:]
                        e2v = e2.rearrange("q (h n) -> q h n", n=512)[:, :, c0:]
                        nc.scalar.activation(e2v, s2v, AF.Exp, scale=SCALE)
                        if jk == 0 and pend is not None:
                            pend()  # previous norm finishers fill this slot
                        sta, spa = jk == 0, jk == nk - 1
                        nc.tensor.matmul(ps_oa[:, c0:512],
                                         va[p][:, jk * 65:(jk + 1) * 65],
                                         e2[:, c0:512], start=sta, stop=spa)
                        nc.tensor.matmul(ps_ob[:, c0:512],
                                         vb[p][:, jk * 128:(jk + 1) * 128],
                                         e2[:, 512 + c0:1024],
                                         start=sta, stop=spa)

                    # softmax denominators: head A's ones-row sits on
                    # partition 64, so it broadcasts via a K=1 PE matmul
                    # (gpsimd partition_broadcast reads absolute partition 0
                    # only); head B's sits on partition 0 and rides the Pool
                    # engine broadcast.
                    rsa = wp.tile([65, 512], F32R, tag="rsa", bufs=2,
                                  name=f"rsa{p}_{jq}")
                    rsb = wp.tile([1, 512], F32R, tag="rsb", bufs=2,
                                  name=f"rsb{p}_{jq}")
                    nc.vector.tensor_copy(rsa[64:65, :], ps_oa[64:65, :])
                    nc.vector.tensor_copy(rsb, ps_ob[0:1, :])

                    def finish():
                        ps_ba = pp.tile([128, 512], F32, tag="proj", bufs=2,
                                        name=f"ba{p}_{jq}")
                        nc.tensor.matmul(ps_ba, ones_sb[64:65, :],
                                         rsa[64:65, :], start=True, stop=True)
                        ps_bb = pp.tile([128, 512], F32, tag="proj", bufs=2,
                                        name=f"bb{p}_{jq}")
                        nc.tensor.matmul(ps_bb, ones_sb[0:1, :], rsb,
                                         start=True, stop=True)
                        bca = wp.tile([128, 512], F32, tag="bca", bufs=2,
                                      name=f"bca{p}_{jq}")
                        bcb = wp.tile([128, 512], F32, tag="bcb", bufs=2,
                                      name=f"bcb{p}_{jq}")
                        nc.vector.reciprocal(bca, ps_ba)
                        nc.vector.tensor_tensor(out=at_sb[0:64, qsl],
                                                in0=ps_oa[0:64, :],
                                                in1=bca[0:64, :],
                                                op=ALU.mult)
                        nc.vector.reciprocal(bcb, ps_bb)
                        nc.vector.tensor_tensor(out=at_sb[64:128, qsl],
                                                in0=ps_ob[64:128, :],
                                                in1=bcb[64:128, :],
                                                op=ALU.mult)
                    return finish

                def out_proj(jn2):
                    o_sb = wp.tile([128, 1024], BF16, tag="osb", bufs=2,
                                   name=f"osb{jn2}")
                    for jo in range(2):
                        ps_o = pp.tile([128, 512], F32, tag="acc", bufs=2,
                                       name=f"po{jn2}_{jo}")
                        for p in range(2):
                            nc.tensor.matmul(
                                ps_o,
                                at_sb[:, p * S + jn2 * 128:
                                      p * S + (jn2 + 1) * 128],
                                wo_sb[:, p * D + jo * 512:
                                      p * D + (jo + 1) * 512],
                                start=(p == 0), stop=(p == 1))
                        osl = o_sb[:, jo * 512:(jo + 1) * 512]
                        nc.vector.tensor_copy(osl, ps_o)
                        nc.sync.dma_start(
                            out=out_d[jn2 * 128:(jn2 + 1) * 128,
                                      jo * 512:(jo + 1) * 512],
                            in_=osl)

                if phases == "p1":
                    for jq in range(NQ):
                        project(jq)
                else:
                    project(0)
                    fin_a = attention(0, 0)
                    for jq in range(NQ):
                        fin_b = attention(1, jq, pend=fin_a)
                        if jq < NQ - 1:
                            project(jq + 1, pend=fin_b)
                        else:
                            fin_b()
                        for jn2 in range(4 * jq, 4 * jq + 4):
                            out_proj(jn2)
                        if jq < NQ - 1:
                            fin_a = attention(0, jq + 1)

                if phases == "p1":  # dummy out write so `out` has a producer
                    dmy = wp.tile([128, 512], BF16, tag="osb", name="dmy")
                    nc.vector.tensor_copy(dmy, qt_sb[:, 0:512])
                    nc.sync.dma_start(out=out_d[0:128, 0:512], in_=dmy)
    nc.finalize()
    return nc


_NC = {}


def _get_nc(loop_iters=None, phases="full"):
    key = (loop_iters, phases)
    if key not in _NC:
        _NC[key] = _build_nc(loop_iters, phases)
    return _NC[key]


def build_in_maps(query, key_in, value, Wq, bq, Wk, bk, Wv, bv, Wo, bo):
    query = np.asarray(query, dtype=np.float32)
    key_in = np.asarray(key_in, dtype=np.float32)
    value = np.asarray(value, dtype=np.float32)
    Wq = np.asarray(Wq, dtype=np.float32)
    Wk = np.asarray(Wk, dtype=np.float32)
    Wv = np.asarray(Wv, dtype=np.float32)
    Wo = np.asarray(Wo, dtype=np.float32)
    bq = np.asarray(bq, dtype=np.float32)
    bk = np.asarray(bk, dtype=np.float32)
    bv = np.asarray(bv, dtype=np.float32)
    bo = np.asarray(bo, dtype=np.float32)

    import ml_dtypes
    bf16 = ml_dtypes.bfloat16
    id128 = np.eye(128, dtype=np.float32).astype(bf16)
    kl = np.arange(128)[:, None]
    jl = np.arange(128)[None, :]
    tri128 = np.where(kl > jl, NEG, 0.0).astype(np.float32).astype(bf16)
    ones_col = np.ones((128, NK, 1), dtype=np.float32).astype(bf16)
    xq = [np.ascontiguousarray(query[s].T).astype(bf16) for s in range(B)]
    xk = [np.ascontiguousarray(key_in[s].T).astype(bf16) for s in range(B)]
    xv = [np.ascontiguousarray(value[s].T).astype(bf16) for s in range(B)]

    in_maps = []
    for c in range(8):
        g, s = c // 2, c % 2
        dsl = slice(g * DL, (g + 1) * DL)
        in_maps.append({
            "xq_t": xq[s],
            "xk_t": xk[s],
            "xv_t": xv[s],
            "wq_t": np.ascontiguousarray(Wq[dsl, :].T).astype(bf16),
            "wk_t": np.ascontiguousarray(Wk[dsl, :].T).astype(bf16),
            "wv_t": np.ascontiguousarray(Wv[dsl, :].T).astype(bf16),
            "wo_t": np.ascontiguousarray(Wo[:, dsl].T).astype(bf16),
            "bq": np.ascontiguousarray(bq[dsl, None]),
            "bk": np.ascontiguousarray(bk[dsl, None]),
            "bv_bc": np.ascontiguousarray(
                np.broadcast_to(bv[None, dsl], (128, DL))),
            "id128": id128,
            "ones66": np.ones((66, 128), dtype=np.float32),
            "tri128": tri128,
            "ones_col": ones_col,
        })
    return in_maps


def kernel(query, key_in, value, Wq, bq, Wk, bk, Wv, bv, Wo, bo):
    bo = np.asarray(bo, dtype=np.float32)
    in_maps = build_in_maps(query, key_in, value, Wq, bq, Wk, bk, Wv, bv, Wo, bo)
    nc = _get_nc()
    res = run_bass_kernel_spmd(nc, in_maps, core_ids=list(range(8)))

    out = np.zeros((B, S, D), dtype=np.float32)
    for c in range(8):
        s = c % 2
        out[s] += np.asarray(res.results[c]["out"], dtype=np.float32)
    out += bo[None, None, :]
    return out


# revision 21
# speedup vs baseline: 1.0457x; 1.0457x over previous
"""Multi-head causal attention (B=2, S=2048, D=1024, H=16) on 8 TRN2 NeuronCores.

Sharding: core c -> (head-group g = c//2 of 4 heads, batch half s = c%2).
Each core computes Q/K/V projections for its 4 heads over its batch element,
causal softmax attention, and a partial output projection (its 256 columns of
Wo). Host sums the 4 per-group partials for each batch element and adds bo.

Device layout notes:
- All activation-side tensors are bf16 (full-rate on the PE, ~4e-3 rel err).
- Activations X are passed pre-transposed (X^T, [D, S]) so every projection
  contracts over the embed dim on the partition axis.
- Scores are computed transposed (S^T [k, q]) so the attention matmul
  (attn @ V) needs no transposes; softmax denominators come from an
  appended ones-column in V, and the normalization divide uses a K=1
  broadcast matmul + vector reciprocal.
- The causal mask on diagonal 128-blocks is applied as a -1e30 PSUM pre-bias
  written by a tiny PE matmul (identity x triangle) before the QK matmuls,
  so the score->exp->attnV chain never touches the vector engine.
- Normalization finishers (broadcast matmul + reciprocal + scale) are
  deferred into the next attention/projection emission point so the PE
  stream is never stalled waiting on the vector engine at q-tile handoffs.
- DMA issue order matches consumption order (tiny bias constants and the
  first wq/xq slices lead the ring) to minimize the pre-first-matmul
  latency paid on every For_i iteration.
- PSUM tags: "proj" (2 banks, Q/K chains + denom broadcasts), "sc"
  (4 banks, V-proj + scores), "acc" (2 banks, attnV accumulators +
  out-proj).
- Output partials are written bf16 (chunked [128,512] DMAs) and summed in
  f32 on the host.
"""

import contextlib
import sys

sys.path.insert(0, "/opt/trn_rl_repo")

import numpy as np

import concourse.bass as bass  # noqa: F401  (bass must import before bacc)
import concourse.mybir as mybir
from concourse import bacc
from concourse.bass_utils import run_bass_kernel_spmd
from concourse.tile import TileContext

F32 = mybir.dt.float32
F32R = mybir.dt.float32r
BF16 = mybir.dt.bfloat16
AF = mybir.ActivationFunctionType
ALU = mybir.AluOpType

B = 2
S = 2048            # sequence per batch element (= rows per core)
D = 1024            # embed dim
H = 16              # total heads
HD = 64             # head dim
DL = 256            # local dims per core (4 heads)
NI = D // 128       # 8 contraction tiles for projections
NQ = S // 512       # 4 query tiles of 512
NK = S // 128       # 16 key tiles of 128
SCALE = HD ** -0.5
NEG = -1e30


def _build_nc(loop_iters=None, phases="full"):
    nc = bacc.Bacc()

    xq_d = nc.declare_dram_parameter("xq_t", [D, S], BF16, isOutput=False)
    xk_d = nc.declare_dram_parameter("xk_t", [D, S], BF16, isOutput=False)
    xv_d = nc.declare_dram_parameter("xv_t", [D, S], BF16, isOutput=False)
    wq_d = nc.declare_dram_parameter("wq_t", [D, DL], BF16, isOutput=False)
    wk_d = nc.declare_dram_parameter("wk_t", [D, DL], BF16, isOutput=False)
    wv_d = nc.declare_dram_parameter("wv_t", [D, DL], BF16, isOutput=False)
    wo_d = nc.declare_dram_parameter("wo_t", [DL, D], BF16, isOutput=False)
    bq_d = nc.declare_dram_parameter("bq", [DL, 1], F32, isOutput=False)
    bk_d = nc.declare_dram_parameter("bk", [DL, 1], F32, isOutput=False)
    bv_d = nc.declare_dram_parameter("bv_bc", [128, DL], F32, isOutput=False)
    id_d = nc.declare_dram_parameter("id128", [128, 128], BF16, isOutput=False)
    tri_d = nc.declare_dram_parameter("tri128", [128, 128], BF16,
                                      isOutput=False)
    on_d = nc.declare_dram_parameter("ones66", [66, 128], F32R, isOutput=False)
    oc_d = nc.declare_dram_parameter("ones_col", [128, NK, 1], BF16,
                                     isOutput=False)
    out_d = nc.declare_dram_parameter("out", [S, D], BF16, isOutput=True)

    with TileContext(nc) as tc:
        with tc.tile_pool(name="const", bufs=1) as cp, \
             tc.tile_pool(name="xpool", bufs=4) as xp, \
             tc.tile_pool(name="work", bufs=3) as wp, \
             tc.tile_pool(name="psum", bufs=8, space="PSUM") as pp:

            ET = mybir.EngineType
            loop_cm = (tc.For_i(0, loop_iters, 1,
                                hint_engines=(ET.PE, ET.DVE, ET.Activation,
                                              ET.SP, ET.Pool))
                       if loop_iters else contextlib.nullcontext())
            with loop_cm:
                # ---- persistent SBUF tensors ----
                wq_sb = cp.tile([128, NI * DL], BF16)
                wk_sb = cp.tile([128, NI * DL], BF16)
                wv_sb = cp.tile([128, NI * DL], BF16)
                wo_sb = cp.tile([128, 2 * D], BF16)
                qt_sb = cp.tile([128, 2 * S], BF16)   # Q^T: pair p cols [p*S:(p+1)*S]
                kt_sb = cp.tile([128, 2 * S], BF16)
                at_sb = cp.tile([128, 2 * S], BF16)   # attn out^T (normalized)
                va0 = cp.tile([128, NK * 65], BF16)   # head A of pair 0, +ones col 64
                va1 = cp.tile([128, NK * 65], BF16)
                vb0 = cp.tile([128, NK * 128], BF16)  # head B: col0=ones, 64:128=V
                vb1 = cp.tile([128, NK * 128], BF16)
                va = [va0, va1]
                vb = [vb0, vb1]
                id_sb = cp.tile([128, 128], BF16)
                ones_sb = cp.tile([66, 128], F32R)
                tri_sb = cp.tile([128, 128], BF16)
                bq_sb = cp.tile([128, 2], F32)
                bk_sb = cp.tile([128, 2], F32)
                bv_sb = cp.tile([128, DL], F32)

                nc.sync.dma_start(
                    out=wq_sb.rearrange("p (a m) -> p a m", m=DL)[:, 0:1],
                    in_=wq_d.rearrange("(a p) m -> p a m", p=128)[:, 0:1])
                nc.sync.dma_start(
                    out=wq_sb.rearrange("p (a m) -> p a m", m=DL)[:, 1:],
                    in_=wq_d.rearrange("(a p) m -> p a m", p=128)[:, 1:])

                # ---- phase 1: projections, emitted per jn and interleaved
                # with attention/out-proj below ----
                def project(jn, pend=None):
                    nsl = slice(jn * 512, (jn + 1) * 512)
                    xq_sl = xp.tile([128, NI * 512], BF16, tag="xq", bufs=2,
                                    name=f"xq_{jn}")
                    xk_sl = xp.tile([128, NI * 512], BF16, tag="xk", bufs=2,
                                    name=f"xk_{jn}")
                    for hf in range(2):  # half-slab DMAs: wave A starts after
                        hi = slice(hf * 4, (hf + 1) * 4)     # the first half
                        hr = slice(hf * 512, (hf + 1) * 512)
                        if jn == 0 and hf == 0:
                            nc.sync.dma_start(
                                out=xq_sl.rearrange(
                                    "p (a n) -> p a n", n=512)[:, 0:1],
                                in_=xq_d[0:128, nsl].rearrange(
                                    "(a p) n -> p a n", p=128))
                            nc.sync.dma_start(
                                out=xq_sl.rearrange(
                                    "p (a n) -> p a n", n=512)[:, 1:4],
                                in_=xq_d[128:512, nsl].rearrange(
                                    "(a p) n -> p a n", p=128))
                        else:
                            nc.sync.dma_start(
                                out=xq_sl.rearrange(
                                    "p (a n) -> p a n", n=512)[:, hi],
                                in_=xq_d[hr, nsl].rearrange(
                                    "(a p) n -> p a n", p=128))
                        if jn == 0 and hf == 0:
                            # first iteration of the ring: interleave the
                            # small constants behind the first q-slab half
                            nc.sync.dma_start(
                                out=wk_sb.rearrange("p (a m) -> p a m", m=DL),
                                in_=wk_d.rearrange("(a p) m -> p a m", p=128))
                        nc.sync.dma_start(
                            out=xk_sl.rearrange("p (a n) -> p a n", n=512)[:, hi],
                            in_=xk_d[hr, nsl].rearrange("(a p) n -> p a n", p=128))
                        if jn == 0 and hf == 0:
                            for p in range(2):
                                nc.sync.dma_start(
                                    out=bq_sb[:, p:p + 1],
                                    in_=bq_d[p * 128:(p + 1) * 128, :])
                                nc.sync.dma_start(
                                    out=bk_sb[:, p:p + 1],
                                    in_=bk_d[p * 128:(p + 1) * 128, :])
                            nc.sync.dma_start(out=bv_sb, in_=bv_d[:])
                            nc.sync.dma_start(out=ones_sb, in_=on_d[:])
                    xq_t = [xq_sl[:, ji * 512:(ji + 1) * 512] for ji in range(NI)]
                    xk_t = [xk_sl[:, ji * 512:(ji + 1) * 512] for ji in range(NI)]

                    if jn == 0:
                        # V path ahead of the remaining constants so wave B
                        # isn't head-of-line blocked on them
                        nc.sync.dma_start(
                            out=wv_sb.rearrange("p (a m) -> p a m", m=DL),
                            in_=wv_d.rearrange("(a p) m -> p a m", p=128))

                    # wave A: Q/K projections as four 512-col chains on the
                    # 2-bank "proj" tag (t-halves sequential), so next-iter
                    # projections never contend with this-iter scores banks.
                    for t in range(2):
                        ps_q = pp.tile([128, 512], F32, tag="proj", bufs=2,
                                       name=f"psq_{jn}_{t}")
                        for ji in range(NI):
                            wsl = slice(ji * DL + t * 128,
                                        ji * DL + (t + 1) * 128)
                            nc.tensor.matmul(ps_q, wq_sb[:, wsl], xq_t[ji],
                                             start=ji == 0, stop=ji == NI - 1)
                        dst = slice(t * S + jn * 512, t * S + (jn + 1) * 512)
                        if t == 0 and pend is not None:
                            pend()  # previous norm finishers ride this stream
                        nc.vector.tensor_scalar(qt_sb[:, dst], ps_q,
                                                bq_sb[:, t:t + 1], None, ALU.add)
                        ps_k = pp.tile([128, 512], F32, tag="proj", bufs=2,
                                       name=f"psk_{jn}_{t}")
                        for ji in range(NI):
                            wsl = slice(ji * DL + t * 128,
                                        ji * DL + (t + 1) * 128)
                            nc.tensor.matmul(ps_k, wk_sb[:, wsl], xk_t[ji],
                                             start=ji == 0, stop=ji == NI - 1)
                        nc.vector.tensor_scalar(kt_sb[:, dst], ps_k,
                                                bk_sb[:, t:t + 1], None, ALU.add)

                    # wave B: V projection, 4 chains of 256 cols in one
                    # 2-bank "sc" tile (sibling chains share a bank via the
                    # per-element overwrite-when-bit-clear PSUM semantics).
                    xv_sl = xp.tile([128, NI * 512], BF16, tag="xv", bufs=2,
                                    name=f"xv_{jn}")
                    for hf in range(2):
                        hi = slice(hf * 4, (hf + 1) * 4)
                        hr = slice(hf * 512, (hf + 1) * 512)
                        nc.sync.dma_start(
                            out=xv_sl.rearrange("p (a n) -> p a n", n=512)[:, hi],
                            in_=xv_d[hr, nsl].rearrange("(a p) n -> p a n", p=128))
                    if jn == 0:
                        # remaining phase-2/3 constants, needed later than xv
                        nc.sync.dma_start(out=id_sb, in_=id_d[:])
                        nc.sync.dma_start(out=tri_sb, in_=tri_d[:])
                        for p in range(2):
                            nc.sync.dma_start(
                                out=va[p].rearrange(
                                    "q (m c) -> q m c", c=65)[:, :, 64:65],
                                in_=oc_d[:])
                            nc.sync.dma_start(
                                out=vb[p].rearrange(
                                    "q (m c) -> q m c", c=128)[:, :, 0:1],
                                in_=oc_d[:])
                        nc.sync.dma_start(
                            out=wo_sb.rearrange("p (a m) -> p a m", m=D),
                            in_=wo_d.rearrange("(a p) m -> p a m", p=128))
                    xv_t = [xv_sl[:, ji * 512:(ji + 1) * 512] for ji in range(NI)]
                    ps_v2 = pp.tile([128, 1024], F32, tag="sc", bufs=2,
                                    name=f"psv_{jn}")
                    ps_v = [ps_v2[:, u * DL:(u + 1) * DL] for u in range(4)]
                    for ji in range(NI):
                        sp = ji == NI - 1
                        for u in range(4):
                            nc.tensor.matmul(
                                ps_v[u],
                                xv_t[ji][:, u * 128:(u + 1) * 128],
                                wv_sb[:, ji * DL:(ji + 1) * DL],
                                start=(ji == 0 and u % 2 == 0), stop=sp)
                    for u in range(4):
                        m = jn * 4 + u
                        for p in range(2):
                            ha = slice(p * 128, p * 128 + 64)
                            hb = slice(p * 128 + 64, p * 128 + 128)
                            nc.vector.tensor_tensor(
                                out=va[p][:, m * 65:m * 65 + 64],
                                in0=ps_v[u][:, ha], in1=bv_sb[:, ha], op=ALU.add)
                            nc.vector.tensor_tensor(
                                out=vb[p][:, m * 128 + 64:m * 128 + 128],
                                in0=ps_v[u][:, hb], in1=bv_sb[:, hb], op=ALU.add)

                # ---- phase 2 + 3: causal attention (head pairs packed on
                # partitions), with the partial output projection interleaved
                # per q-tile ----
                def attention(p, jq, pend=None):
                    qsl = slice(p * S + jq * 512, p * S + (jq + 1) * 512)
                    nk = 4 * jq + 4
                    ps_oa = pp.tile([65, 512], F32, tag="acc", bufs=2,
                                    name=f"oa{p}_{jq}")
                    ps_ob = pp.tile([128, 512], F32, tag="acc", bufs=2,
                                    name=f"ob{p}_{jq}")
                    for jk in range(nk):
                        d = jk - 4 * jq
                        c0 = 128 * d if d > 0 else 0  # first causally-valid col
                        ksl = slice(p * S + jk * 128, p * S + (jk + 1) * 128)
                        qsl_v = slice(p * S + jq * 512 + c0,
                                      p * S + (jq + 1) * 512)
                        ps_s2 = pp.tile([128, 1024], F32, tag="sc", bufs=2,
                                        name=f"s2{p}_{jq}_{jk}")
                        st = d < 0
                        if d >= 0:
                            # diagonal block: pre-bias the masked triangle
                            # (rows k > cols q within the 128-window) with
                            # -1e30 via a tiny PE matmul; start=True clears
                            # the bank so the QK matmuls overwrite the rest.
                            nc.tensor.matmul(ps_s2[:, c0:c0 + 128],
                                             id_sb, tri_sb,
                                             start=True, stop=True)
                            nc.tensor.matmul(ps_s2[:, 512 + c0:512 + c0 + 128],
                                             id_sb, tri_sb,
                                             start=True, stop=True)
                        nc.tensor.matmul(ps_s2[:, c0:512], kt_sb[0:64, ksl],
                                         qt_sb[0:64, qsl_v],
                                         start=st, stop=True,
                                         tile_position=(0, 0),
                                         skip_group_check=not st)
                        nc.tensor.matmul(ps_s2[:, 512 + c0:1024],
                                         kt_sb[64:128, ksl],
                                         qt_sb[64:128, qsl_v],
                                         start=st, stop=True,
                                         tile_position=(64, 0),
                                         skip_group_check=not st)
                        e2 = wp.tile([128, 1024], BF16, tag="e2", bufs=8,
                                     name=f"e2{p}_{jq}_{jk}")
                        s2v = ps_s2.rearrange("q (h n) -> q h n", n=512)[:, :, c0:]
                        e2v = e2.rearrange("q (h n) -> q h n", n=512)[:, :, c0:]
                        nc.scalar.activation(e2v, s2v, AF.Exp, scale=SCALE)
                        if jk == 0 and pend is not None:
                            pend()  # previous norm finishers fill this slot
                        sta, spa = jk == 0, jk == nk - 1
                        nc.tensor.matmul(ps_oa[:, c0:512],
                                         va[p][:, jk * 65:(jk + 1) * 65],
                                         e2[:, c0:512], start=sta, stop=spa)
                        nc.tensor.matmul(ps_ob[:, c0:512],
                                         vb[p][:, jk * 128:(jk + 1) * 128],
                                         e2[:, 512 + c0:1024],
                                         start=sta, stop=spa)

                    # softmax denominators: head A's ones-row sits on
                    # partition 64, so it broadcasts via a K=1 PE matmul
                    # (gpsimd partition_broadcast reads absolute partition 0
                    # only); head B's sits on partition 0 and rides the Pool
                    # engine broadcast.
                    rsa = wp.tile([65, 512], F32R, tag="rsa", bufs=2,
                                  name=f"rsa{p}_{jq}")
                    rsb = wp.tile([1, 512], F32R, tag="rsb", bufs=2,
                                  name=f"rsb{p}_{jq}")
                    nc.vector.tensor_copy(rsa[64:65, :], ps_oa[64:65, :])
                    nc.vector.tensor_copy(rsb, ps_ob[0:1, :])

                    def finish():
                        ps_ba = pp.tile([128, 512], F32, tag="proj", bufs=2,
                                        name=f"ba{p}_{jq}")
                        nc.tensor.matmul(ps_ba, ones_sb[64:65, :],
                                         rsa[64:65, :], start=True, stop=True)
                        ps_bb = pp.tile([128, 512], F32, tag="proj", bufs=2,
                                        name=f"bb{p}_{jq}")
                        nc.tensor.matmul(ps_bb, ones_sb[0:1, :], rsb,
                                         start=True, stop=True)
                        bca = wp.tile([128, 512], F32, tag="bca", bufs=2,
                                      name=f"bca{p}_{jq}")
                        bcb = wp.tile([128, 512], F32, tag="bcb", bufs=2,
                                      name=f"bcb{p}_{jq}")
                        nc.vector.reciprocal(bca, ps_ba)
                        nc.vector.tensor_tensor(out=at_sb[0:64, qsl],
                                                in0=ps_oa[0:64, :],
                                                in1=bca[0:64, :],
                                                op=ALU.mult)
                        nc.vector.reciprocal(bcb, ps_bb)
                        nc.vector.tensor_tensor(out=at_sb[64:128, qsl],
                                                in0=ps_ob[64:128, :],
                                                in1=bcb[64:128, :],
                                                op=ALU.mult)
                    return finish

                def out_proj(jn2):
                    o_sb = wp.tile([128, 1024], BF16, tag="osb", bufs=2,
                                   name=f"osb{jn2}")
                    for jo in range(2):
                        ps_o = pp.tile([128, 512], F32, tag="acc", bufs=2,
                                       name=f"po{jn2}_{jo}")
                        for p in range(2):
                            nc.tensor.matmul(
                                ps_o,
                                at_sb[:, p * S + jn2 * 128:
                                      p * S + (jn2 + 1) * 128],
                                wo_sb[:, p * D + jo * 512:
                                      p * D + (jo + 1) * 512],
                                start=(p == 0), stop=(p == 1))
                        osl = o_sb[:, jo * 512:(jo + 1) * 512]
                        nc.vector.tensor_copy(osl, ps_o)
                        nc.sync.dma_start(
                            out=out_d[jn2 * 128:(jn2 + 1) * 128,
                                      jo * 512:(jo + 1) * 512],
                            in_=osl)

                if phases == "p1":
                    for jq in range(NQ):
                        project(jq)
                else:
                    project(0)
                    fin_a = attention(0, 0)
                    for jq in range(NQ):
                        fin_b = attention(1, jq, pend=fin_a)
                        if jq < NQ - 1:
                            project(jq + 1, pend=fin_b)
                        else:
                            fin_b()
                        for jn2 in range(4 * jq, 4 * jq + 4):
                            out_proj(jn2)
                        if jq < NQ - 1:
                            fin_a = attention(0, jq + 1)

                if phases == "p1":  # dummy out write so `out` has a producer
                    dmy = wp.tile([128, 512], BF16, tag="osb", name="dmy")
                    nc.vector.tensor_copy(dmy, qt_sb[:, 0:512])
                    nc.sync.dma_start(out=out_d[0:128, 0:512], in_=dmy)
    nc.finalize()
    return nc


_NC = {}


def _get_nc(loop_iters=None, phases="full"):
    key = (loop_iters, phases)
    if key not in _NC:
        _NC[key] = _build_nc(loop_iters, phases)
    return _NC[key]


def build_in_maps(query, key_in, value, Wq, bq, Wk, bk, Wv, bv, Wo, bo):
    query = np.asarray(query, dtype=np.float32)
    key_in = np.asarray(key_in, dtype=np.float32)
    value = np.asarray(value, dtype=np.float32)
    Wq = np.asarray(Wq, dtype=np.float32)
    Wk = np.asarray(Wk, dtype=np.float32)
    Wv = np.asarray(Wv, dtype=np.float32)
    Wo = np.asarray(Wo, dtype=np.float32)
    bq = np.asarray(bq, dtype=np.float32)
    bk = np.asarray(bk, dtype=np.float32)
    bv = np.asarray(bv, dtype=np.float32)
    bo = np.asarray(bo, dtype=np.float32)

    import ml_dtypes
    bf16 = ml_dtypes.bfloat16
    id128 = np.eye(128, dtype=np.float32).astype(bf16)
    kl = np.arange(128)[:, None]
    jl = np.arange(128)[None, :]
    tri128 = np.where(kl > jl, NEG, 0.0).astype(np.float32).astype(bf16)
    ones_col = np.ones((128, NK, 1), dtype=np.float32).astype(bf16)
    xq = [np.ascontiguousarray(query[s].T).astype(bf16) for s in range(B)]
    xk = [np.ascontiguousarray(key_in[s].T).astype(bf16) for s in range(B)]
    xv = [np.ascontiguousarray(value[s].T).astype(bf16) for s in range(B)]

    in_maps = []
    for c in range(8):
        g, s = c // 2, c % 2
        dsl = slice(g * DL, (g + 1) * DL)
        in_maps.append({
            "xq_t": xq[s],
            "xk_t": xk[s],
            "xv_t": xv[s],
            "wq_t": np.ascontiguousarray(Wq[dsl, :].T).astype(bf16),
            "wk_t": np.ascontiguousarray(Wk[dsl, :].T).astype(bf16),
            "wv_t": np.ascontiguousarray(Wv[dsl, :].T).astype(bf16),
            "wo_t": np.ascontiguousarray(Wo[:, dsl].T).astype(bf16),
            "bq": np.ascontiguousarray(bq[dsl, None]),
            "bk": np.ascontiguousarray(bk[dsl, None]),
            "bv_bc": np.ascontiguousarray(
                np.broadcast_to(bv[None, dsl], (128, DL))),
            "id128": id128,
            "ones66": np.ones((66, 128), dtype=np.float32),
            "tri128": tri128,
            "ones_col": ones_col,
        })
    return in_maps


def kernel(query, key_in, value, Wq, bq, Wk, bk, Wv, bv, Wo, bo):
    bo = np.asarray(bo, dtype=np.float32)
    in_maps = build_in_maps(query, key_in, value, Wq, bq, Wk, bk, Wv, bv, Wo, bo)
    nc = _get_nc()
    res = run_bass_kernel_spmd(nc, in_maps, core_ids=list(range(8)))

    out = np.zeros((B, S, D), dtype=np.float32)
    for c in range(8):
        s = c % 2
        out[s] += np.asarray(res.results[c]["out"], dtype=np.float32)
    out += bo[None, None, :]
    return out
